# revision 3
# baseline (speedup 1.0000x reference)
"""Causal attention kernel for Trainium2, 8 NeuronCores — fp16 + fp8 tiers.

Problem: x[4,2048,2048] @ Wq/Wk/Wv[2048,2048] -> causal softmax attention.

Sharding: 2 cores per batch; each core owns 1024 query rows as global
512-row chunks {0,3} (even cores) / {1,2} (odd cores). Slot 0 = c_lo chunk,
slot 1 = c_hi chunk. Across the pair, slot 0 covers global rows 0-1023 and
slot 1 covers rows 1024-2047 — that boundary is also the precision tier:

 - slot-0 path (rows < 1024, concentrated-softmax rows): fp16 everywhere
   (projections, scores, probabilities, AV).
 - slot-1 path (rows >= 1024): q/k staged e4m3, scores via fp8 DoubleRow
   matmuls (2x PE rate); V/AV stay fp16 but the c_hi V projection runs as
   fp8 DoubleRow (noise tolerable; validated to rel-err ~9e-3 vs 2e-2 gate).

fp8 scaling: W8 = e4m3(W*64); q/k staged at x32 natural scale; exp scale
folds the 1/1024. Scores computed transposed (S^T = K Q^T), softmax without
max-subtraction (fp16 range suffices: exp(s) <= ~250), row sums via
ones-matmuls, normalization after AV.
"""

import math

import numpy as np
import ml_dtypes

import concourse.bass as bass
import concourse.mybir as mybir
import concourse.tile as tile
from concourse import bacc
from concourse.bass import ds, ts
from concourse.bass_utils import run_bass_kernel_spmd

B, S, D = 4, 2048, 2048
P = 128
DC = D // P          # 16 contraction chunks
SB = S // P          # 16 key blocks
QROWS = 1024         # query rows per core
NCORES = 8
INV_SQRT_D = 1.0 / math.sqrt(D)
W8_SCALE = 64.0      # host pre-scale folded into fp8 W
QK_SCALE = 32.0      # q/k staged in e4m3 at this scale

# gathered key-block position -> true 512-chunk (pair-rank order, all cores)
POS2TRUE = [0, 3, 1, 2]
# key-block positions processed by query slot 0
SLOT0_POS = [0, 1, 2, 3, 8, 9, 10, 11]
S0IDX = {pos: j for j, pos in enumerate(SLOT0_POS)}
PAIRS = [[0, 1], [2, 3], [4, 5], [6, 7]]

F32 = mybir.dt.float32
F16 = mybir.dt.float16
E4 = mybir.dt.float8e4
DR = mybir.MatmulPerfMode.DoubleRow
Exp = mybir.ActivationFunctionType.Exp
Copy = mybir.ActivationFunctionType.Copy

_CACHED_NC = None


def build_nc():
    global _CACHED_NC
    if _CACHED_NC is not None:
        return _CACHED_NC
    nc = bacc.Bacc(trn_type="TRN2", target_bir_lowering=False, debug=False,
                   num_devices=NCORES)

    xt16_d = nc.dram_tensor("xt16", [D, 512], F16, kind="ExternalInput")
    xt8_d = nc.dram_tensor("xt8", [D, 512], E4, kind="ExternalInput")
    wq16_d = nc.dram_tensor("wq16", [DC, P, DC, P], F16, kind="ExternalInput")
    wk16_d = nc.dram_tensor("wk16", [DC, P, DC, P], F16, kind="ExternalInput")
    wv16_d = nc.dram_tensor("wv16", [4, 2, P, 8, 512], F16, kind="ExternalInput")
    wq8_d = nc.dram_tensor("wq8", [DC, P, DC, P], E4, kind="ExternalInput")
    wk8_d = nc.dram_tensor("wk8", [DC, P, DC, P], E4, kind="ExternalInput")
    wv8_d = nc.dram_tensor("wv8", [4, 2, P, 8, 512], E4, kind="ExternalInput")
    mk_d = nc.dram_tensor("masks", [P, 24, 512], F16, kind="ExternalInput")
    out_d = nc.dram_tensor("out", [QROWS, D], F32, kind="ExternalOutput")

    with tile.TileContext(nc) as tc:
        with (
            tc.tile_pool(name="dram", bufs=1, space="DRAM") as dpool,
            tc.tile_pool(name="ps", bufs=8, space="PSUM") as ps_all,
        ):
            qT16 = dpool.tile([P, DC, 512], F16, tag="qT16")
            qT8 = dpool.tile([P, DC, 512], E4, tag="qT8")
            kT16_own = dpool.tile([4, P, DC, P], F16, tag="kT16o")
            kT8lo_own = dpool.tile([4, P, DC, P], E4, tag="kT8lo")
            kT8hi_own = dpool.tile([4, P, DC, P], E4, tag="kT8hi")
            kg16 = dpool.tile([2, 4, P, DC, P], F16, tag="kg16")
            kg8lo = dpool.tile([2, 4, P, DC, P], E4, tag="kg8lo")
            kg8hi = dpool.tile([2, 4, P, DC, P], E4, tag="kg8hi")
            vv_own = [dpool.tile([2, P, D], F16, tag=f"vvo{g}",
                                 name=f"vvo{g}") for g in range(4)]
            vgs = [dpool.tile([2, 2, P, D], F16, tag=f"vg{g}",
                              name=f"vg{g}") for g in range(4)]

            # ---------------- phase 1: projections ----------------
            with tc.tile_pool(name="xt", bufs=1) as xt_pool:
                xts16 = xt_pool.tile([P, DC, 512], F16, tag="xt16")
                xts8 = xt_pool.tile([P, DC, 512], E4, tag="xt8")

                # --- K (s0 half in fp16, staged both f16 + e4m3)
                with (
                    tc.tile_pool(name="wk16", bufs=16) as wk16_pool,
                    tc.tile_pool(name="wk8", bufs=16) as wk8_pool,
                    tc.tile_pool(name="st", bufs=10) as st_pool,
                    tc.tile_pool(name="st8", bufs=8) as st8_pool,
                ):
                    def load_w16(pool, dram, m, name):
                        wt = pool.tile([P, DC, P], F16, tag="w", name=name)
                        nc.sync.dma_start(wt[:, :8, :], dram.ap()[m][:, :8, :])
                        nc.sync.dma_start(wt[:, 8:, :], dram.ap()[m][:, 8:, :])
                        return wt

                    def load_w8(pool, dram, m, name):
                        wt = pool.tile([P, DC, P], E4, tag="w8", name=name)
                        nc.sync.dma_start(wt[:, :8, :], dram.ap()[m][:, :8, :])
                        nc.sync.dma_start(wt[:, 8:, :], dram.ap()[m][:, 8:, :])
                        return wt

                    wk16_pre = [load_w16(wk16_pool, wk16_d, 0, "wk16p0")]
                    for dc in range(DC):
                        nc.sync.dma_start(xts16[:, dc, :],
                                          xt16_d.ap()[ds(dc * P, P), :])
                    wk16_pre += [load_w16(wk16_pool, wk16_d, m, f"wk16p{m}")
                                 for m in range(1, DC)]
                    for dc in range(DC):
                        nc.sync.dma_start(xts8[:, dc, :],
                                            xt8_d.ap()[ds(dc * P, P), :])
                    wk8_pre = [load_w8(wk8_pool, wk8_d, m, f"wk8p{m}")
                               for m in range(DC)]

                    # K s0: fp16 matmuls, dual staging (f16 for slot-0
                    # scores, e4m3 x32 for slot-1 scores)
                    for m in range(DC):
                        wt = wk16_pre[m]
                        ps = ps_all.tile([P, 512], F32, tag="ps")
                        for dc in range(DC):
                            nc.tensor.matmul(
                                ps[:], lhsT=wt[:, dc, :],
                                rhs=xts16[:, dc, :],
                                start=(dc == 0), stop=(dc == DC - 1),
                            )
                        st = st_pool.tile([P, 512], F16, tag="st")
                        nc.vector.tensor_copy(st[:], ps[:])
                        st8 = st8_pool.tile([P, 512], E4, tag="st8")
                        nc.scalar.activation(st8[:], ps[:], Copy,
                                             scale=QK_SCALE)
                        for j in range(4):
                            nc.gpsimd.dma_start(kT16_own[j, :, m, :],
                                                st[:, ts(j, P)])
                            nc.scalar.dma_start(kT8lo_own[j, :, m, :],
                                                st8[:, ts(j, P)])
                    nc.gpsimd.collective_compute(
                        "AllGather", mybir.AluOpType.bypass,
                        replica_groups=PAIRS,
                        ins=[kT16_own.opt()], outs=[kg16.opt()],
                    )
                    nc.gpsimd.collective_compute(
                        "AllGather", mybir.AluOpType.bypass,
                        replica_groups=PAIRS,
                        ins=[kT8lo_own.opt()], outs=[kg8lo.opt()],
                    )

                    # K s1: fp8 DoubleRow matmuls (W8 carries x64; stage
                    # e4m3 at x32 via 0.5 scale)
                    for m in range(DC):
                        wt8 = wk8_pre[m]
                        ps = ps_all.tile([P, 512], F32, tag="ps")
                        for c in range(8):
                            nc.tensor.matmul(
                                ps[:], lhsT=wt8[:, 2*c:2*c+2, :],
                                rhs=xts8[:, 2*c:2*c+2, :],
                                start=(c == 0), stop=(c == 7),
                                perf_mode=DR,
                            )
                        st8 = st8_pool.tile([P, 512], E4, tag="st8")
                        nc.scalar.activation(st8[:], ps[:], Copy,
                                             scale=QK_SCALE / W8_SCALE)
                        for j in range(4):
                            nc.scalar.dma_start(kT8hi_own[j, :, m, :],
                                                st8[:, ts(j, P)])
                    nc.gpsimd.collective_compute(
                        "AllGather", mybir.AluOpType.bypass,
                        replica_groups=PAIRS,
                        ins=[kT8hi_own.opt()], outs=[kg8hi.opt()],
                    )

                # --- V (s0-3 fp16; s4-7 fp8 DR -> fp16 staging)
                with (
                    tc.tile_pool(name="wv16", bufs=8) as wv16_pool,
                    tc.tile_pool(name="wv8", bufs=8) as wv8_pool,
                    tc.tile_pool(name="stv", bufs=8) as stv_pool,
                ):
                    wv16_pre = []
                    wv8_pre = []
                    for n in range(4):
                        for hb in range(2):
                            wvt = wv16_pool.tile([P, 8, 512], F16, tag="wv",
                                                 name=f"wv{n}{hb}")
                            nc.sync.dma_start(wvt[:, :4, :],
                                              wv16_d.ap()[n, hb][:, :4, :])
                            nc.sync.dma_start(wvt[:, 4:, :],
                                              wv16_d.ap()[n, hb][:, 4:, :])
                            wv16_pre.append(wvt)
                            wv8t = wv8_pool.tile([P, 8, 512], E4, tag="wv8",
                                                 name=f"wv8{n}{hb}")
                            nc.sync.dma_start(wv8t[:, :4, :],
                                                wv8_d.ap()[n, hb][:, :4, :])
                            nc.sync.dma_start(wv8t[:, 4:, :],
                                                wv8_d.ap()[n, hb][:, 4:, :])
                            wv8_pre.append(wv8t)
                    for s in range(8):
                        for n in range(4):
                            ps = ps_all.tile([P, 512], F32, tag="ps")
                            if s < 4:
                                wva = wv16_pre[2 * n]
                                wvb = wv16_pre[2 * n + 1]
                                for dc in range(DC):
                                    w = wva if dc < 8 else wvb
                                    nc.tensor.matmul(
                                        ps[:],
                                        lhsT=xts16[:, dc, ts(s, P)],
                                        rhs=w[:, dc % 8, :],
                                        start=(dc == 0), stop=(dc == DC - 1),
                                    )
                                sv = stv_pool.tile([P, 512], F16, tag="sv")
                                nc.vector.tensor_copy(sv[:], ps[:])
                            else:
                                wva = wv8_pre[2 * n]
                                wvb = wv8_pre[2 * n + 1]
                                for c in range(8):
                                    w = wva if c < 4 else wvb
                                    nc.tensor.matmul(
                                        ps[:],
                                        lhsT=xts8[:, 2*c:2*c+2, ts(s - 4, P)],
                                        rhs=w[:, (2*c) % 8:(2*c) % 8 + 2, :],
                                        start=(c == 0), stop=(c == 7),
                                        perf_mode=DR,
                                    )
                                sv = stv_pool.tile([P, 512], F16, tag="sv")
                                nc.scalar.activation(sv[:], ps[:], Copy,
                                                     scale=1.0 / W8_SCALE)
                            nc.gpsimd.dma_start(
                                vv_own[s // 2][s % 2, :, ts(n, 512)], sv[:])
                        if s % 2 == 1:
                            g = s // 2
                            nc.gpsimd.collective_compute(
                                "AllGather", mybir.AluOpType.bypass,
                                replica_groups=PAIRS,
                                ins=[vv_own[g].opt()],
                                outs=[vgs[g].opt()],
                            )

                # --- Q (slot0 fp16; slot1 fp8 DR -> e4m3 staging)
                with (
                    tc.tile_pool(name="wq16", bufs=6) as wq16_pool,
                    tc.tile_pool(name="wq8", bufs=6) as wq8_pool,
                    tc.tile_pool(name="stq", bufs=6) as stq_pool,
                    tc.tile_pool(name="stq8", bufs=6) as stq8_pool,
                ):
                    def load_wq16(m):
                        wt = wq16_pool.tile([P, DC, P], F16, tag="w",
                                            name=f"wq16_{m}")
                        nc.sync.dma_start(wt[:, :8, :], wq16_d.ap()[m][:, :8, :])
                        nc.sync.dma_start(wt[:, 8:, :], wq16_d.ap()[m][:, 8:, :])
                        return wt

                    def load_wq8(m):
                        wt = wq8_pool.tile([P, DC, P], E4, tag="w8",
                                           name=f"wq8_{m}")
                        nc.sync.dma_start(wt[:, :8, :], wq8_d.ap()[m][:, :8, :])
                        nc.sync.dma_start(wt[:, 8:, :], wq8_d.ap()[m][:, 8:, :])
                        return wt

                    PRE = 3
                    wq16_tiles = {m: load_wq16(m) for m in range(PRE)}
                    wq8_tiles = {m: load_wq8(m) for m in range(PRE)}
                    for m in range(DC):
                        if m + PRE < DC:
                            wq16_tiles[m + PRE] = load_wq16(m + PRE)
                            wq8_tiles[m + PRE] = load_wq8(m + PRE)
                        wt = wq16_tiles.pop(m)
                        ps = ps_all.tile([P, 512], F32, tag="ps")
                        for dc in range(DC):
                            nc.tensor.matmul(
                                ps[:], lhsT=wt[:, dc, :],
                                rhs=xts16[:, dc, :],
                                start=(dc == 0), stop=(dc == DC - 1),
                            )
                        st = stq_pool.tile([P, 512], F16, tag="st")
                        nc.vector.tensor_copy(st[:], ps[:])
                        nc.gpsimd.dma_start(qT16[:, m, :], st[:])

                        wt8 = wq8_tiles.pop(m)
                        ps = ps_all.tile([P, 512], F32, tag="ps")
                        for c in range(8):
                            nc.tensor.matmul(
                                ps[:], lhsT=wt8[:, 2*c:2*c+2, :],
                                rhs=xts8[:, 2*c:2*c+2, :],
                                start=(c == 0), stop=(c == 7),
                                perf_mode=DR,
                            )
                        st8 = stq8_pool.tile([P, 512], E4, tag="st8")
                        nc.scalar.activation(st8[:], ps[:], Copy,
                                             scale=QK_SCALE / W8_SCALE)
                        nc.scalar.dma_start(qT8[:, m, :], st8[:])

            # ---------------- phase 2: attention ----------------
            with (
                tc.tile_pool(name="pt", bufs=1) as pt_pool,
                tc.tile_pool(name="mk", bufs=1) as mk_pool,
                tc.tile_pool(name="vb", bufs=2) as vb_pool,
                tc.tile_pool(name="kt16", bufs=5) as kt16_pool,
                tc.tile_pool(name="kt8", bufs=6) as kt8_pool,
                tc.tile_pool(name="qt", bufs=1) as qt_pool,
                tc.tile_pool(name="qt8", bufs=1) as qt8_pool,
                tc.tile_pool(name="one", bufs=1) as one_pool,
                tc.tile_pool(name="sc", bufs=4) as sc_pool,
                tc.tile_pool(name="ob", bufs=4) as ob_pool,
            ):
                mk = mk_pool.tile([P, 24, 512], F16, tag="mk")
                for j in range(3):
                    nc.gpsimd.dma_start(mk[:, ts(j, 8), :],
                                        mk_d.ap()[:, ts(j, 8), :])
                ones = one_pool.tile([P, 1], F16, tag="ones")
                nc.vector.memset(ones[:], 1.0)
                # pt index: slot0 j -> key pos SLOT0_POS[j]; slot1 kb -> 8+kb
                pt = pt_pool.tile([P, 24, 512], F16, tag="pt")

                qt16 = qt_pool.tile([P, DC, 512], F16, tag="qt16")
                for j in range(4):
                    nc.sync.dma_start(qt16[:, ts(j, 4), :],
                                      qT16[:, ts(j, 4), :])
                qt8 = qt8_pool.tile([P, DC, 512], E4, tag="qt8")
                for j in range(2):
                    nc.sync.dma_start(qt8[:, ts(j, 8), :],
                                      qT8[:, ts(j, 8), :])

                # --- slot-0 scores (fp16): 8 key positions
                for pos in SLOT0_POS:
                    kt_t = kt16_pool.tile([P, DC, P], F16, tag="kt",
                                          name=f"kt16_{pos}")
                    ksrc = kg16[0 if pos < 8 else 1, pos % 4]
                    for j in range(4):
                        nc.sync.dma_start(kt_t[:, ts(j, 4), :],
                                          ksrc[:, ts(j, 4), :])
                    ps = ps_all.tile([P, 512], F32, tag="ps")
                    for dc in range(DC):
                        nc.tensor.matmul(
                            ps[:], lhsT=kt_t[:, dc, :],
                            rhs=qt16[:, dc, :],
                            start=(dc == 0), stop=(dc == DC - 1),
                        )
                    nc.scalar.activation(pt[:, S0IDX[pos], :], ps[:], Exp,
                                         scale=INV_SQRT_D)

                # --- slot-1 scores (fp8 DoubleRow): all 16 key positions
                for pos in range(SB):
                    o = pos % 8
                    kt_t = kt8_pool.tile([P, DC, P], E4, tag="kt8",
                                         name=f"kt8_{pos}")
                    ksrc = (kg8lo if o < 4 else kg8hi)[pos // 8, o % 4]
                    for j in range(2):
                        nc.sync.dma_start(kt_t[:, ts(j, 8), :],
                                          ksrc[:, ts(j, 8), :])
                    ps = ps_all.tile([P, 512], F32, tag="ps")
                    for c in range(8):
                        nc.tensor.matmul(
                            ps[:], lhsT=kt_t[:, 2*c:2*c+2, :],
                            rhs=qt8[:, 2*c:2*c+2, :],
                            start=(c == 0), stop=(c == 7),
                            perf_mode=DR,
                        )
                    nc.scalar.activation(pt[:, 8 + pos, :], ps[:], Exp,
                                         scale=INV_SQRT_D / (QK_SCALE ** 2))

                # --- V tiles; masks; AV; normalize
                vbig = []
                for half in range(2):
                    vb = vb_pool.tile([P, 8, D], F16, tag="vb",
                                      name=f"vb{half}")
                    for j in range(8):
                        nc.gpsimd.dma_start(vb[:, j, :], vgs[j // 2][half, j % 2])
                    vbig.append(vb)

                def vt_ap(pos):
                    return vbig[pos // 8][:, pos % 8, :]

                for j in range(24):
                    nc.vector.tensor_mul(pt[:, j, :], pt[:, j, :], mk[:, j, :])

                for slot in range(2):
                    for qs in range(4):
                        # per-qs causal union over the pair: key blocks that
                        # are fully masked on BOTH cores are skipped
                        if slot == 0:
                            kpos = [0, 1, 2, 3] + list(range(8, 9 + qs))
                            idx = [S0IDX[p] for p in kpos]
                        else:
                            kpos = ([0, 1, 2, 3] + list(range(4, 5 + qs))
                                    + list(range(8, 16)))
                            idx = [8 + p for p in kpos]
                        plt = ps_all.tile([P, 512], F32, tag="ps", name="pl")
                        pl = plt[:, :1]
                        for i, j in enumerate(idx):
                            nc.tensor.matmul(
                                pl[:], lhsT=pt[:, j, ts(qs, P)],
                                rhs=ones[:],
                                start=(i == 0), stop=(i == len(idx) - 1),
                            )
                        rl = sc_pool.tile([P, 1], F32, tag="rl")
                        nc.vector.reciprocal(rl[:], pl[:])
                        for n in range(4):
                            pav = ps_all.tile([P, 512], F32, tag="ps",
                                              name="pav")
                            for i, j in enumerate(idx):
                                nc.tensor.matmul(
                                    pav[:], lhsT=pt[:, j, ts(qs, P)],
                                    rhs=vt_ap(kpos[i])[:, ts(n, 512)],
                                    start=(i == 0),
                                    stop=(i == len(idx) - 1),
                                )
                            ob = ob_pool.tile([P, 512], F32, tag="ob")
                            if n % 2 == 0:
                                nc.vector.tensor_scalar_mul(ob[:], pav[:],
                                                            rl[:])
                            else:
                                nc.scalar.activation(ob[:], pav[:], Copy,
                                                     scale=rl[:])
                            nc.sync.dma_start(
                                out_d.ap()[ds(slot * 512 + qs * P, P),
                                           ts(n, 512)],
                                ob[:],
                            )

    nc.compile()
    _CACHED_NC = nc
    return nc


def _host_prep(x, Wq, Wk, Wv):
    """Build per-core input maps (host-side layout prep)."""
    E4np = ml_dtypes.float8_e4m3

    def wqk_prep(W, dt, scale=1.0):
        return np.ascontiguousarray(
            (W * scale).reshape(DC, P, DC, P).transpose(2, 1, 0, 3)).astype(dt)

    def wv_prep(W, dt, scale=1.0):
        return np.ascontiguousarray(
            (W * scale).reshape(2, 8, P, 4, 512).transpose(3, 0, 2, 1, 4)
        ).astype(dt)

    wq16_h = wqk_prep(Wq, np.float16)
    wk16_h = wqk_prep(Wk, np.float16)
    wq8_h = wqk_prep(Wq, E4np, W8_SCALE)
    wk8_h = wqk_prep(Wk, E4np, W8_SCALE)
    wv16_h = wv_prep(Wv, np.float16)
    wv8_h = wv_prep(Wv, E4np, W8_SCALE)

    k_in_block = np.arange(P, dtype=np.int64)[:, None]           # [P, 1]
    q_in_chunk = np.arange(512, dtype=np.int64)[None, :]         # [1, 512]

    def build_masks(h):
        c_lo, c_hi = h, 3 - h
        masks = np.zeros((P, 24, 512), dtype=np.float16)
        for j, pos in enumerate(SLOT0_POS):
            tkb = POS2TRUE[pos // 4] * 4 + pos % 4
            masks[:, j, :] = (tkb * P + k_in_block) <= (c_lo * 512 + q_in_chunk)
        for pos in range(SB):
            tkb = POS2TRUE[pos // 4] * 4 + pos % 4
            masks[:, 8 + pos, :] = (tkb * P + k_in_block) <= (c_hi * 512 + q_in_chunk)
        return masks

    mask_h = [build_masks(0), build_masks(1)]

    in_maps = []
    for core in range(NCORES):
        b, h = divmod(core, 2)
        c_lo, c_hi = h, 3 - h
        xt = x[b].T                                               # [D, S] view
        xt_lo = np.ascontiguousarray(xt[:, c_lo * 512:(c_lo + 1) * 512])
        xt_hi = np.ascontiguousarray(xt[:, c_hi * 512:(c_hi + 1) * 512])
        in_maps.append({
            "xt16": xt_lo.astype(np.float16),
            "xt8": xt_hi.astype(E4np),
            "wq16": wq16_h, "wk16": wk16_h, "wv16": wv16_h,
            "wq8": wq8_h, "wk8": wk8_h, "wv8": wv8_h,
            "masks": mask_h[h],
        })
    return in_maps


def run(x, Wq, Wk, Wv, trace=False):
    x = np.asarray(x, dtype=np.float32)
    Wq = np.asarray(Wq, dtype=np.float32)
    Wk = np.asarray(Wk, dtype=np.float32)
    Wv = np.asarray(Wv, dtype=np.float32)
    nc = build_nc()
    in_maps = _host_prep(x, Wq, Wk, Wv)
    res = run_bass_kernel_spmd(nc, in_maps, core_ids=list(range(NCORES)),
                               trace=trace)
    out = np.empty((B, S, D), dtype=np.float32)
    for core in range(NCORES):
        b, h = divmod(core, 2)
        c_lo, c_hi = h, 3 - h
        o = res.results[core]["out"]
        out[b, c_lo * 512:(c_lo + 1) * 512] = o[:512]
        out[b, c_hi * 512:(c_hi + 1) * 512] = o[512:]
    return out, res


def kernel(x, Wq, Wk, Wv):
    out, _ = run(x, Wq, Wk, Wv)
    return out


if __name__ == "__main__":
    build_nc()
    print("build + compile OK")


# revision 14
# speedup vs baseline: 1.0966x; 1.0966x over previous
"""Causal attention kernel for Trainium2, 8 NeuronCores — fp16 + fp8 tiers.

Problem: x[4,2048,2048] @ Wq/Wk/Wv[2048,2048] -> causal softmax attention.

Sharding: 2 cores per batch; each core owns 1024 query rows as global
512-row chunks {0,3} (even cores) / {1,2} (odd cores). Slot 0 = c_lo chunk,
slot 1 = c_hi chunk. Across the pair, slot 0 covers global rows 0-1023 and
slot 1 covers rows 1024-2047 — that boundary is also the precision tier:

 - slot-0 path (rows < 1024, concentrated-softmax rows): fp16 everywhere.
 - slot-1 path (rows >= 1024): q/k staged e4m3, scores via fp8 DoubleRow
   matmuls (2x PE rate); AV stays fp16 but the c_hi V projection runs as
   fp8 DoubleRow (validated rel-err ~9e-3 vs the 2e-2 gate).

fp8 scaling: W8 = e4m3(W*64); q/k staged at x32; exp scale folds 1/1024.
Scores computed transposed (S^T = K Q^T), softmax without max-subtraction,
row sums via ones-matmuls, normalization after AV.

Feeding the PE (the p-state trap): all W/x loads are ONE-SHOT (no ring
reuse -> no semaphore-gated DMA queues), spread over sync/scalar/gpsimd.
K^T staging is one DMA per m-tile ([P, DC, 512] block layout); gathered
K arrives in phase 2 as 6 whole-rank contiguous loads that score matmuls
slice in SBUF. Collectives are batched into 3 AllGathers. q is staged
straight into phase-2 SBUF tiles.
"""

import math

import numpy as np
import ml_dtypes

import concourse.bass as bass
import concourse.mybir as mybir
import concourse.tile as tile
from concourse import bacc
from concourse.bass import ds, ts
from concourse.bass_utils import run_bass_kernel_spmd

B, S, D = 4, 2048, 2048
P = 128
DC = D // P          # 16 contraction chunks
SB = S // P          # 16 key blocks
QROWS = 1024         # query rows per core
NCORES = 8
INV_SQRT_D = 1.0 / math.sqrt(D)
W8_SCALE = 64.0      # host pre-scale folded into fp8 W
QK_SCALE = 32.0      # q/k staged in e4m3 at this scale

# gathered key-block position -> true 512-chunk (pair-rank order, all cores)
POS2TRUE = [0, 3, 1, 2]
# key-block positions processed by query slot 0
SLOT0_POS = [0, 1, 2, 3, 8, 9, 10, 11]
S0IDX = {pos: j for j, pos in enumerate(SLOT0_POS)}
PAIRS = [[0, 1], [2, 3], [4, 5], [6, 7]]

F32 = mybir.dt.float32
F16 = mybir.dt.float16
E4 = mybir.dt.float8e4
DR = mybir.MatmulPerfMode.DoubleRow
Exp = mybir.ActivationFunctionType.Exp
Copy = mybir.ActivationFunctionType.Copy

_CACHED_NC = None


def build_nc():
    global _CACHED_NC
    if _CACHED_NC is not None:
        return _CACHED_NC
    nc = bacc.Bacc(trn_type="TRN2", target_bir_lowering=False, debug=False,
                   num_devices=NCORES)

    xt16_d = nc.dram_tensor("xt16", [P, DC, 512], F16, kind="ExternalInput")
    xt8_d = nc.dram_tensor("xt8", [P, DC, 512], E4, kind="ExternalInput")
    wq16_d = nc.dram_tensor("wq16", [DC, P, DC, P], F16, kind="ExternalInput")
    wk16_d = nc.dram_tensor("wk16", [DC, P, DC, P], F16, kind="ExternalInput")
    wv16_d = nc.dram_tensor("wv16", [4, 2, P, 8, 512], F16, kind="ExternalInput")
    wq8_d = nc.dram_tensor("wq8", [DC, P, DC, P], E4, kind="ExternalInput")
    wk8_d = nc.dram_tensor("wk8", [DC, P, DC, P], E4, kind="ExternalInput")
    wv8_d = nc.dram_tensor("wv8", [4, 2, P, 8, 512], E4, kind="ExternalInput")
    mk_d = nc.dram_tensor("masks", [P, 24, 512], F16, kind="ExternalInput")
    out_d = nc.dram_tensor("out", [QROWS, D], F32, kind="ExternalOutput")

    with tile.TileContext(nc) as tc:
        with (
            tc.tile_pool(name="dram", bufs=1, space="DRAM") as dpool,
            tc.tile_pool(name="ps", bufs=8, space="PSUM") as ps_all,
        ):
            # staging: last dim 512 = 4 key blocks x 128, one DMA per m
            kT16_own = dpool.tile([P, DC, 512], F16, tag="kT16o")
            kT8_own = dpool.tile([2, P, DC, 512], E4, tag="kT8o")
            kg16 = dpool.tile([2, P, DC, 512], F16, tag="kg16")
            kg8 = dpool.tile([2, 2, P, DC, 512], E4, tag="kg8")
            vv_all = dpool.tile([4, 2, P, D], F16, tag="vva")
            vgs_all = dpool.tile([2, 4, 2, P, D], F16, tag="vgs")

            with (
                tc.tile_pool(name="qt", bufs=1) as qt_pool,
                tc.tile_pool(name="qt8", bufs=1) as qt8_pool,
            ):
                # phase-2 q tiles, staged directly from projection PSUM
                qt16 = qt_pool.tile([P, DC, 512], F16, tag="qt16")
                qt8 = qt8_pool.tile([P, DC, 512], E4, tag="qt8")

                # ---------------- phase 1: projections ----------------
                with (
                    tc.tile_pool(name="xt", bufs=1) as xt_pool,
                    tc.tile_pool(name="wv16", bufs=8) as wv16_pool,
                ):
                    xts16 = xt_pool.tile([P, DC, 512], F16, tag="xt16")
                    xts8 = xt_pool.tile([P, DC, 512], E4, tag="xt8")
                    wv16_pre = []

                    # --- K phase: all loads one-shot
                    with (
                        tc.tile_pool(name="wk16", bufs=14) as wk16_pool,
                        tc.tile_pool(name="wk8", bufs=16) as wk8_pool,
                        tc.tile_pool(name="st", bufs=4) as st_pool,
                        tc.tile_pool(name="st8", bufs=2) as st8_pool,
                    ):
                        wk16_pre = []
                        wt = wk16_pool.tile([P, DC, P], F16, tag="w",
                                            name="wk16p0")
                        nc.sync.dma_start(xts16[:], xt16_d.ap())
                        nc.sync.dma_start(wt[:], wk16_d.ap()[0])
                        wk16_pre.append(wt)
                        for m in range(1, DC - 2):
                            wt = wk16_pool.tile([P, DC, P], F16, tag="w",
                                                name=f"wk16p{m}")
                            nc.sync.dma_start(wt[:], wk16_d.ap()[m])
                            wk16_pre.append(wt)
                        nc.sync.dma_start(xts8[:], xt8_d.ap())
                        for m in range(DC - 2, DC):
                            wt = wk16_pool.tile([P, DC, P], F16, tag="w",
                                                name=f"wk16p{m}")
                            nc.sync.dma_start(wt[:], wk16_d.ap()[m])
                            wk16_pre.append(wt)
                        for j in range(8):
                            wvt = wv16_pool.tile([P, 8, 512], F16, tag="wv",
                                                 name=f"wv{j}")
                            nc.sync.dma_start(wvt[:],
                                              wv16_d.ap()[j // 2, j % 2])
                            wv16_pre.append(wvt)
                        wk8_pre = []

                        # K s0: fp16 matmuls, dual staging (f16 + e4m3 x32)
                        for m in range(DC):
                            # one-shot fp8 K weights ride gpsimd, 1/iter
                            wt8 = wk8_pool.tile([P, DC, P], E4, tag="w8",
                                                name=f"wk8p{m}")
                            nc.gpsimd.dma_start(wt8[:], wk8_d.ap()[m])
                            wk8_pre.append(wt8)

                            wt = wk16_pre[m]
                            ps = ps_all.tile([P, 512], F32, tag="ps")
                            for dc in range(DC):
                                nc.tensor.matmul(
                                    ps[:], lhsT=wt[:, dc, :],
                                    rhs=xts16[:, dc, :],
                                    start=(dc == 0), stop=(dc == DC - 1),
                                )
                            st = st_pool.tile([P, 512], F16, tag="st")
                            nc.vector.tensor_copy(st[:], ps[:])
                            st8 = st8_pool.tile([P, 512], E4, tag="st8")
                            nc.scalar.activation(st8[:], ps[:], Copy,
                                                 scale=QK_SCALE)
                            nc.gpsimd.dma_start(kT16_own[:, m, :], st[:])
                            nc.scalar.dma_start(kT8_own[0][:, m, :], st8[:])
                        nc.gpsimd.collective_compute(
                            "AllGather", mybir.AluOpType.bypass,
                            replica_groups=PAIRS,
                            ins=[kT16_own.opt()], outs=[kg16.opt()],
                        )

                        # K s1: fp8 DoubleRow (W8 carries x64 -> 0.5 scale)
                        for m in range(DC):
                            wt8 = wk8_pre[m]
                            ps = ps_all.tile([P, 512], F32, tag="ps")
                            for c in range(8):
                                nc.tensor.matmul(
                                    ps[:], lhsT=wt8[:, 2*c:2*c+2, :],
                                    rhs=xts8[:, 2*c:2*c+2, :],
                                    start=(c == 0), stop=(c == 7),
                                    perf_mode=DR,
                                )
                            st8 = st8_pool.tile([P, 512], E4, tag="st8")
                            nc.scalar.activation(st8[:], ps[:], Copy,
                                                 scale=QK_SCALE / W8_SCALE)
                            nc.scalar.dma_start(kT8_own[1][:, m, :], st8[:])
                        nc.gpsimd.collective_compute(
                            "AllGather", mybir.AluOpType.bypass,
                            replica_groups=PAIRS,
                            ins=[kT8_own.opt()], outs=[kg8.opt()],
                        )

                    # --- V phase (s0-3 fp16; s4-7 fp8 DR -> fp16 staging)
                    with (
                        tc.tile_pool(name="wv8", bufs=8) as wv8_pool,
                        tc.tile_pool(name="stv", bufs=8) as stv_pool,
                        tc.tile_pool(name="wq16", bufs=6) as wq16_pool,
                        tc.tile_pool(name="wq8", bufs=6) as wq8_pool,
                    ):
                        wv8_pre = []
                        for j in range(8):
                            wvt = wv8_pool.tile([P, 8, 512], E4, tag="wv8",
                                                name=f"wv8{j}")
                            nc.sync.dma_start(wvt[:],
                                              wv8_d.ap()[j // 2, j % 2])
                            wv8_pre.append(wvt)

                        wq16_tiles = {}
                        wq8_tiles = {}

                        def load_wq(m):
                            wt = wq16_pool.tile([P, DC, P], F16, tag="w",
                                                name=f"wq16_{m}")
                            nc.sync.dma_start(wt[:], wq16_d.ap()[m])
                            wq16_tiles[m] = wt
                            wt8 = wq8_pool.tile([P, DC, P], E4, tag="w8",
                                                name=f"wq8_{m}")
                            nc.scalar.dma_start(wt8[:], wq8_d.ap()[m])
                            wq8_tiles[m] = wt8

                        for s in range(8):
                            if s < 5:
                                load_wq(s)
                            for n in range(4):
                                ps = ps_all.tile([P, 512], F32, tag="ps")
                                if s < 4:
                                    wva = wv16_pre[2 * n]
                                    wvb = wv16_pre[2 * n + 1]
                                    for dc in range(DC):
                                        w = wva if dc < 8 else wvb
                                        nc.tensor.matmul(
                                            ps[:],
                                            lhsT=xts16[:, dc, ts(s, P)],
                                            rhs=w[:, dc % 8, :],
                                            start=(dc == 0),
                                            stop=(dc == DC - 1),
                                        )
                                    sv = stv_pool.tile([P, 512], F16,
                                                       tag="sv")
                                    nc.vector.tensor_copy(sv[:], ps[:])
                                else:
                                    wva = wv8_pre[2 * n]
                                    wvb = wv8_pre[2 * n + 1]
                                    for c in range(8):
                                        w = wva if c < 4 else wvb
                                        nc.tensor.matmul(
                                            ps[:],
                                            lhsT=xts8[:, 2*c:2*c+2,
                                                      ts(s - 4, P)],
                                            rhs=w[:, (2*c) % 8:(2*c) % 8 + 2,
                                                  :],
                                            start=(c == 0), stop=(c == 7),
                                            perf_mode=DR,
                                        )
                                    sv = stv_pool.tile([P, 512], F16,
                                                       tag="sv")
                                    nc.scalar.activation(sv[:], ps[:], Copy,
                                                         scale=1.0 / W8_SCALE)
                                nc.gpsimd.dma_start(
                                    vv_all[s // 2, s % 2, :, ts(n, 512)],
                                    sv[:])
                        nc.gpsimd.collective_compute(
                            "AllGather", mybir.AluOpType.bypass,
                            replica_groups=PAIRS,
                            ins=[vv_all.opt()], outs=[vgs_all.opt()],
                        )

                        # --- Q (slot0 fp16 -> qt16; slot1 fp8 DR -> qt8)
                        for m in range(DC):
                            if m + 5 < DC:
                                load_wq(m + 5)
                            wt = wq16_tiles.pop(m)
                            ps = ps_all.tile([P, 512], F32, tag="ps")
                            for dc in range(DC):
                                nc.tensor.matmul(
                                    ps[:], lhsT=wt[:, dc, :],
                                    rhs=xts16[:, dc, :],
                                    start=(dc == 0), stop=(dc == DC - 1),
                                )
                            nc.vector.tensor_copy(qt16[:, m, :], ps[:])

                            wt8 = wq8_tiles.pop(m)
                            ps = ps_all.tile([P, 512], F32, tag="ps")
                            for c in range(8):
                                nc.tensor.matmul(
                                    ps[:], lhsT=wt8[:, 2*c:2*c+2, :],
                                    rhs=xts8[:, 2*c:2*c+2, :],
                                    start=(c == 0), stop=(c == 7),
                                    perf_mode=DR,
                                )
                            nc.scalar.activation(qt8[:, m, :], ps[:], Copy,
                                                 scale=QK_SCALE / W8_SCALE)

                # ---------------- phase 2: attention ----------------
                with (
                    tc.tile_pool(name="pt", bufs=1) as pt_pool,
                    tc.tile_pool(name="mk", bufs=1) as mk_pool,
                    tc.tile_pool(name="vb", bufs=2) as vb_pool,
                    tc.tile_pool(name="ktg16", bufs=2) as ktg16_pool,
                    tc.tile_pool(name="ktg8", bufs=2) as ktg8_pool,
                    tc.tile_pool(name="one", bufs=1) as one_pool,
                    tc.tile_pool(name="sc", bufs=4) as sc_pool,
                    tc.tile_pool(name="ob", bufs=3) as ob_pool,
                ):
                    # whole-rank gathered K tiles; score matmuls slice them
                    ktg16 = []
                    for r in range(2):
                        kt = ktg16_pool.tile([P, DC, 512], F16, tag="ktg",
                                             name=f"ktg16_{r}")
                        nc.sync.dma_start(kt[:], kg16[r])
                        ktg16.append(kt)

                    mk = mk_pool.tile([P, 24, 512], F16, tag="mk")
                    vbig = []
                    for half in range(2):
                        vb = vb_pool.tile([P, 8, D], F16, tag="vb",
                                          name=f"vb{half}")
                        for j in range(8):
                            nc.gpsimd.dma_start(
                                vb[:, j, :], vgs_all[half, j // 2, j % 2])
                        vbig.append(vb)
                    ones = one_pool.tile([P, 1], F16, tag="ones")
                    nc.vector.memset(ones[:], 1.0)
                    # pt: slot0 j -> key pos SLOT0_POS[j]; slot1 kb -> 8+kb
                    pt = pt_pool.tile([P, 24, 512], F16, tag="pt")

                    # --- slot-0 scores (fp16): 8 key positions
                    for pos in SLOT0_POS:
                        kt_t = ktg16[pos // 8]
                        jb = pos % 4
                        ps = ps_all.tile([P, 512], F32, tag="ps")
                        for dc in range(DC):
                            nc.tensor.matmul(
                                ps[:],
                                lhsT=kt_t[:, dc, ds(jb * P, P)],
                                rhs=qt16[:, dc, :],
                                start=(dc == 0), stop=(dc == DC - 1),
                            )
                        nc.scalar.activation(pt[:, S0IDX[pos], :], ps[:],
                                             Exp, scale=INV_SQRT_D)

                    # --- slot-1 scores (fp8 DoubleRow): 16 key positions,
                    # lo tiles (kg8[.,0]) first, then hi tiles
                    ktg8 = {}
                    for r, h in ((0, 0), (1, 0), (0, 1), (1, 1)):
                        kt = ktg8_pool.tile([P, DC, 512], E4, tag="ktg8",
                                            name=f"ktg8_{r}{h}")
                        nc.sync.dma_start(kt[:], kg8[r, h])
                        ktg8[(r, h)] = kt
                    nc.sync.dma_start(mk[:], mk_d.ap())
                    for pos in ([0, 1, 2, 3, 8, 9, 10, 11]
                                + [4, 5, 6, 7, 12, 13, 14, 15]):
                        o = pos % 8
                        kt_t = ktg8[(pos // 8, 0 if o < 4 else 1)]
                        jb = o % 4
                        ps = ps_all.tile([P, 512], F32, tag="ps")
                        for c in range(8):
                            nc.tensor.matmul(
                                ps[:],
                                lhsT=kt_t[:, 2*c:2*c+2, ds(jb * P, P)],
                                rhs=qt8[:, 2*c:2*c+2, :],
                                start=(c == 0), stop=(c == 7),
                                perf_mode=DR,
                            )
                        nc.scalar.activation(pt[:, 8 + pos, :], ps[:], Exp,
                                             scale=INV_SQRT_D / (QK_SCALE ** 2))

                    def vt_ap(pos):
                        return vbig[pos // 8][:, pos % 8, :]

                    for j in range(24):
                        nc.vector.tensor_mul(pt[:, j, :], pt[:, j, :],
                                             mk[:, j, :])

                    for slot in range(2):
                        for qs in range(4):
                            # per-qs causal union over the pair
                            if slot == 0:
                                kpos = [0, 1, 2, 3] + list(range(8, 9 + qs))
                                idx = [S0IDX[p] for p in kpos]
                            else:
                                kpos = ([0, 1, 2, 3] + list(range(4, 5 + qs))
                                        + list(range(8, 16)))
                                idx = [8 + p for p in kpos]
                            plt = ps_all.tile([P, 512], F32, tag="ps",
                                              name="pl")
                            pl = plt[:, :1]
                            for i, j in enumerate(idx):
                                nc.tensor.matmul(
                                    pl[:], lhsT=pt[:, j, ts(qs, P)],
                                    rhs=ones[:],
                                    start=(i == 0), stop=(i == len(idx) - 1),
                                )
                            rl = sc_pool.tile([P, 1], F32, tag="rl")
                            nc.vector.reciprocal(rl[:], pl[:])
                            for n in range(4):
                                pav = ps_all.tile([P, 512], F32, tag="ps",
                                                  name="pav")
                                for i, j in enumerate(idx):
                                    nc.tensor.matmul(
                                        pav[:], lhsT=pt[:, j, ts(qs, P)],
                                        rhs=vt_ap(kpos[i])[:, ts(n, 512)],
                                        start=(i == 0),
                                        stop=(i == len(idx) - 1),
                                    )
                                ob = ob_pool.tile([P, 512], F32, tag="ob")
                                if n % 2 == 0:
                                    nc.vector.tensor_scalar_mul(ob[:],
                                                                pav[:],
                                                                rl[:])
                                else:
                                    nc.scalar.activation(ob[:], pav[:],
                                                         Copy, scale=rl[:])
                                eng = nc.sync if n % 2 == 0 else nc.scalar
                                eng.dma_start(
                                    out_d.ap()[ds(slot * 512 + qs * P, P),
                                               ts(n, 512)],
                                    ob[:],
                                )

    nc.compile()
    _CACHED_NC = nc
    return nc


def _host_prep(x, Wq, Wk, Wv):
    """Build per-core input maps (host-side layout prep)."""
    E4np = ml_dtypes.float8_e4m3

    def wqk_prep(W, dt, scale=1.0):
        return np.ascontiguousarray(
            (W * scale).reshape(DC, P, DC, P).transpose(2, 1, 0, 3)).astype(dt)

    def wv_prep(W, dt, scale=1.0):
        return np.ascontiguousarray(
            (W * scale).reshape(2, 8, P, 4, 512).transpose(3, 0, 2, 1, 4)
        ).astype(dt)

    wq16_h = wqk_prep(Wq, np.float16)
    wk16_h = wqk_prep(Wk, np.float16)
    wq8_h = wqk_prep(Wq, E4np, W8_SCALE)
    wk8_h = wqk_prep(Wk, E4np, W8_SCALE)
    wv16_h = wv_prep(Wv, np.float16)
    wv8_h = wv_prep(Wv, E4np, W8_SCALE)

    k_in_block = np.arange(P, dtype=np.int64)[:, None]           # [P, 1]
    q_in_chunk = np.arange(512, dtype=np.int64)[None, :]         # [1, 512]

    def build_masks(h):
        c_lo, c_hi = h, 3 - h
        masks = np.zeros((P, 24, 512), dtype=np.float16)
        for j, pos in enumerate(SLOT0_POS):
            tkb = POS2TRUE[pos // 4] * 4 + pos % 4
            masks[:, j, :] = (tkb * P + k_in_block) <= (c_lo * 512 + q_in_chunk)
        for pos in range(SB):
            tkb = POS2TRUE[pos // 4] * 4 + pos % 4
            masks[:, 8 + pos, :] = (tkb * P + k_in_block) <= (c_hi * 512 + q_in_chunk)
        return masks

    mask_h = [build_masks(0), build_masks(1)]

    def xt_prep(xt_slice, dt):
        # [D, 512] -> [P, DC, 512] (partition-major, one DMA per core)
        return np.ascontiguousarray(
            xt_slice.reshape(DC, P, 512).transpose(1, 0, 2)).astype(dt)

    in_maps = []
    for core in range(NCORES):
        b, h = divmod(core, 2)
        c_lo, c_hi = h, 3 - h
        xt = x[b].T                                               # [D, S] view
        in_maps.append({
            "xt16": xt_prep(xt[:, c_lo * 512:(c_lo + 1) * 512], np.float16),
            "xt8": xt_prep(xt[:, c_hi * 512:(c_hi + 1) * 512], E4np),
            "wq16": wq16_h, "wk16": wk16_h, "wv16": wv16_h,
            "wq8": wq8_h, "wk8": wk8_h, "wv8": wv8_h,
            "masks": mask_h[h],
        })
    return in_maps


def run(x, Wq, Wk, Wv, trace=False):
    x = np.asarray(x, dtype=np.float32)
    Wq = np.asarray(Wq, dtype=np.float32)
    Wk = np.asarray(Wk, dtype=np.float32)
    Wv = np.asarray(Wv, dtype=np.float32)
    nc = build_nc()
    in_maps = _host_prep(x, Wq, Wk, Wv)
    res = run_bass_kernel_spmd(nc, in_maps, core_ids=list(range(NCORES)),
                               trace=trace)
    out = np.empty((B, S, D), dtype=np.float32)
    for core in range(NCORES):
        b, h = divmod(core, 2)
        c_lo, c_hi = h, 3 - h
        o = res.results[core]["out"]
        out[b, c_lo * 512:(c_lo + 1) * 512] = o[:512]
        out[b, c_hi * 512:(c_hi + 1) * 512] = o[512:]
    return out, res


def kernel(x, Wq, Wk, Wv):
    out, _ = run(x, Wq, Wk, Wv)
    return out


if __name__ == "__main__":
    build_nc()
    print("build + compile OK")


# revision 16
# speedup vs baseline: 1.1627x; 1.0602x over previous
"""Causal attention kernel for Trainium2, 8 NeuronCores — fp16 + fp8 tiers.

Problem: x[4,2048,2048] @ Wq/Wk/Wv[2048,2048] -> causal softmax attention.

Sharding: 2 cores per batch; each core owns 1024 query rows as global
512-row chunks {0,3} (even cores) / {1,2} (odd cores). Slot 0 = c_lo chunk,
slot 1 = c_hi chunk. Across the pair, slot 0 covers global rows 0-1023 and
slot 1 covers rows 1024-2047 — that boundary is also the precision tier:

 - slot-0 path (rows < 1024, concentrated-softmax rows): fp16 everywhere.
 - slot-1 path (rows >= 1024): q/k staged e4m3, scores via fp8 DoubleRow
   matmuls (2x PE rate); AV stays fp16 but the c_hi V projection runs as
   fp8 DoubleRow (validated rel-err ~9e-3 vs the 2e-2 gate).

fp8 scaling: W8 = e4m3(W*64); q/k staged at x32; exp scale folds 1/1024.
Scores computed transposed (S^T = K Q^T), softmax without max-subtraction,
row sums via ones-matmuls, normalization after AV.

Feeding the PE (the p-state trap): all W/x loads are ONE-SHOT (no ring
reuse -> no semaphore-gated DMA queues), spread over sync/scalar/gpsimd.
K^T staging is one DMA per m-tile ([P, DC, 512] block layout); gathered
K arrives in phase 2 as 6 whole-rank contiguous loads that score matmuls
slice in SBUF. Collectives are batched into 3 AllGathers. q is staged
straight into phase-2 SBUF tiles.
"""

import math

import numpy as np
import ml_dtypes

import concourse.bass as bass
import concourse.mybir as mybir
import concourse.tile as tile
from concourse import bacc
from concourse.bass import ds, ts
from concourse.bass_utils import run_bass_kernel_spmd

B, S, D = 4, 2048, 2048
P = 128
DC = D // P          # 16 contraction chunks
SB = S // P          # 16 key blocks
QROWS = 1024         # query rows per core
NCORES = 8
INV_SQRT_D = 1.0 / math.sqrt(D)
W8_SCALE = 64.0      # host pre-scale folded into fp8 W
QK_SCALE = 32.0      # q/k staged in e4m3 at this scale

# gathered key-block position -> true 512-chunk (pair-rank order, all cores)
POS2TRUE = [0, 3, 1, 2]
# key-block positions processed by query slot 0
SLOT0_POS = [0, 1, 2, 3, 8, 9, 10, 11]
S0IDX = {pos: j for j, pos in enumerate(SLOT0_POS)}
PAIRS = [[0, 1], [2, 3], [4, 5], [6, 7]]

F32 = mybir.dt.float32
F16 = mybir.dt.float16
E4 = mybir.dt.float8e4
DR = mybir.MatmulPerfMode.DoubleRow
Exp = mybir.ActivationFunctionType.Exp
Copy = mybir.ActivationFunctionType.Copy

_CACHED_NC = None


def build_nc():
    global _CACHED_NC
    if _CACHED_NC is not None:
        return _CACHED_NC
    nc = bacc.Bacc(trn_type="TRN2", target_bir_lowering=False, debug=False,
                   num_devices=NCORES)

    xt16_d = nc.dram_tensor("xt16", [P, DC, 512], F16, kind="ExternalInput")
    xt8_d = nc.dram_tensor("xt8", [P, DC, 512], E4, kind="ExternalInput")
    wq16_d = nc.dram_tensor("wq16", [DC, P, DC, P], F16, kind="ExternalInput")
    wk16_d = nc.dram_tensor("wk16", [DC, P, DC, P], F16, kind="ExternalInput")
    wv16_d = nc.dram_tensor("wv16", [4, 2, P, 8, 512], F16, kind="ExternalInput")
    wq8_d = nc.dram_tensor("wq8", [DC, P, DC, P], E4, kind="ExternalInput")
    wk8_d = nc.dram_tensor("wk8", [DC, P, DC, P], E4, kind="ExternalInput")
    wv8_d = nc.dram_tensor("wv8", [4, 2, P, 8, 512], E4, kind="ExternalInput")
    mk_d = nc.dram_tensor("masks", [P, 24, 512], F16, kind="ExternalInput")
    out_d = nc.dram_tensor("out", [QROWS, D], F32, kind="ExternalOutput")

    with tile.TileContext(nc) as tc:
        with (
            tc.tile_pool(name="dram", bufs=1, space="DRAM") as dpool,
            tc.tile_pool(name="ps", bufs=8, space="PSUM") as ps_all,
        ):
            # staging: last dim 512 = 4 key blocks x 128, one DMA per m
            kT16_own = dpool.tile([P, DC, 512], F16, tag="kT16o")
            kT8_own = dpool.tile([2, P, DC, 512], E4, tag="kT8o")
            kg16 = dpool.tile([2, P, DC, 512], F16, tag="kg16")
            kg8 = dpool.tile([2, 2, P, DC, 512], E4, tag="kg8")
            vv_all = dpool.tile([4, 2, P, D], F16, tag="vva")
            vgs_all = dpool.tile([2, 4, 2, P, D], F16, tag="vgs")

            with (
                tc.tile_pool(name="qt", bufs=1) as qt_pool,
                tc.tile_pool(name="qt8", bufs=1) as qt8_pool,
            ):
                # phase-2 q tiles, staged directly from projection PSUM
                qt16 = qt_pool.tile([P, DC, 512], F16, tag="qt16")
                qt8 = qt8_pool.tile([P, DC, 512], E4, tag="qt8")

                # ---------------- phase 1: projections ----------------
                with (
                    tc.tile_pool(name="xt", bufs=1) as xt_pool,
                    tc.tile_pool(name="wv16", bufs=8) as wv16_pool,
                ):
                    xts16 = xt_pool.tile([P, DC, 512], F16, tag="xt16")
                    xts8 = xt_pool.tile([P, DC, 512], E4, tag="xt8")
                    wv16_pre = []

                    # --- K phase: all loads one-shot
                    with (
                        tc.tile_pool(name="wk16", bufs=12) as wk16_pool,
                        tc.tile_pool(name="wk8", bufs=16) as wk8_pool,
                        tc.tile_pool(name="st", bufs=3) as st_pool,
                        tc.tile_pool(name="st8", bufs=2) as st8_pool,
                        tc.tile_pool(name="k81", bufs=1) as k81_pool,
                    ):
                        wk16_pre = []
                        wt = wk16_pool.tile([P, DC, P], F16, tag="w",
                                            name="wk16p0")
                        for j in range(4):
                            nc.sync.dma_start(xts16[:, ts(j, 4), :],
                                              xt16_d.ap()[:, ts(j, 4), :])
                        nc.sync.dma_start(wt[:], wk16_d.ap()[0])
                        wk16_pre.append(wt)
                        for m in range(1, DC - 4):
                            wt = wk16_pool.tile([P, DC, P], F16, tag="w",
                                                name=f"wk16p{m}")
                            nc.sync.dma_start(wt[:], wk16_d.ap()[m])
                            wk16_pre.append(wt)
                        for j in range(2):
                            nc.sync.dma_start(xts8[:, ts(j, 8), :],
                                              xt8_d.ap()[:, ts(j, 8), :])
                        for m in range(DC - 4, DC):
                            wt = wk16_pool.tile([P, DC, P], F16, tag="w",
                                                name=f"wk16p{m}")
                            nc.sync.dma_start(wt[:], wk16_d.ap()[m])
                            wk16_pre.append(wt)
                        for j in range(8):
                            wvt = wv16_pool.tile([P, 8, 512], F16, tag="wv",
                                                 name=f"wv{j}")
                            nc.sync.dma_start(wvt[:],
                                              wv16_d.ap()[j // 2, j % 2])
                            wv16_pre.append(wvt)
                        wk8_pre = []

                        # K s0: fp16 matmuls, dual staging (f16 + e4m3 x32)
                        for m in range(DC):
                            # one-shot fp8 K weights ride gpsimd, 1/iter
                            wt8 = wk8_pool.tile([P, DC, P], E4, tag="w8",
                                                name=f"wk8p{m}")
                            nc.gpsimd.dma_start(wt8[:], wk8_d.ap()[m])
                            wk8_pre.append(wt8)

                            wt = wk16_pre[m]
                            ps = ps_all.tile([P, 512], F32, tag="ps")
                            for dc in range(DC):
                                nc.tensor.matmul(
                                    ps[:], lhsT=wt[:, dc, :],
                                    rhs=xts16[:, dc, :],
                                    start=(dc == 0), stop=(dc == DC - 1),
                                )
                            st = st_pool.tile([P, 512], F16, tag="st")
                            nc.vector.tensor_copy(st[:], ps[:])
                            st8 = st8_pool.tile([P, 512], E4, tag="st8")
                            nc.scalar.activation(st8[:], ps[:], Copy,
                                                 scale=QK_SCALE)
                            nc.gpsimd.dma_start(kT16_own[:, m, :], st[:])
                            nc.scalar.dma_start(kT8_own[0][:, m, :], st8[:])
                        nc.gpsimd.collective_compute(
                            "AllGather", mybir.AluOpType.bypass,
                            replica_groups=PAIRS,
                            ins=[kT16_own.opt()], outs=[kg16.opt()],
                        )

                        # K s1: fp8 DoubleRow (W8 carries x64 -> 0.5
                        # scale). Staged fully in SBUF: per-m DMAs here
                        # would stall behind the kg16 gather's bandwidth.
                        kT8s1 = k81_pool.tile([P, DC, 512], E4, tag="k81",
                                              name="kT8s1")
                        for m in range(DC):
                            wt8 = wk8_pre[m]
                            ps = ps_all.tile([P, 512], F32, tag="ps")
                            for c in range(8):
                                nc.tensor.matmul(
                                    ps[:], lhsT=wt8[:, 2*c:2*c+2, :],
                                    rhs=xts8[:, 2*c:2*c+2, :],
                                    start=(c == 0), stop=(c == 7),
                                    perf_mode=DR,
                                )
                            nc.scalar.activation(kT8s1[:, m, :], ps[:], Copy,
                                                 scale=QK_SCALE / W8_SCALE)
                        nc.gpsimd.dma_start(kT8_own[1], kT8s1[:])
                        nc.gpsimd.collective_compute(
                            "AllGather", mybir.AluOpType.bypass,
                            replica_groups=PAIRS,
                            ins=[kT8_own.opt()], outs=[kg8.opt()],
                        )

                    # --- V phase (s0-3 fp16; s4-7 fp8 DR -> fp16 staging)
                    with (
                        tc.tile_pool(name="wv8", bufs=8) as wv8_pool,
                        tc.tile_pool(name="stv", bufs=8) as stv_pool,
                        tc.tile_pool(name="wq16", bufs=8) as wq16_pool,
                        tc.tile_pool(name="wq8", bufs=6) as wq8_pool,
                    ):
                        wv8_pre = []
                        for j in range(8):
                            wvt = wv8_pool.tile([P, 8, 512], E4, tag="wv8",
                                                name=f"wv8{j}")
                            nc.sync.dma_start(wvt[:],
                                              wv8_d.ap()[j // 2, j % 2])
                            wv8_pre.append(wvt)

                        wq16_tiles = {}
                        wq8_tiles = {}

                        def load_wq(m):
                            wt = wq16_pool.tile([P, DC, P], F16, tag="w",
                                                name=f"wq16_{m}")
                            nc.sync.dma_start(wt[:], wq16_d.ap()[m])
                            wq16_tiles[m] = wt
                            wt8 = wq8_pool.tile([P, DC, P], E4, tag="w8",
                                                name=f"wq8_{m}")
                            nc.scalar.dma_start(wt8[:], wq8_d.ap()[m])
                            wq8_tiles[m] = wt8

                        for s in range(8):
                            if s < 7:
                                load_wq(s)
                            for n in range(4):
                                ps = ps_all.tile([P, 512], F32, tag="ps")
                                if s < 4:
                                    wva = wv16_pre[2 * n]
                                    wvb = wv16_pre[2 * n + 1]
                                    for dc in range(DC):
                                        w = wva if dc < 8 else wvb
                                        nc.tensor.matmul(
                                            ps[:],
                                            lhsT=xts16[:, dc, ts(s, P)],
                                            rhs=w[:, dc % 8, :],
                                            start=(dc == 0),
                                            stop=(dc == DC - 1),
                                        )
                                    sv = stv_pool.tile([P, 512], F16,
                                                       tag="sv")
                                    nc.vector.tensor_copy(sv[:], ps[:])
                                else:
                                    wva = wv8_pre[2 * n]
                                    wvb = wv8_pre[2 * n + 1]
                                    for c in range(8):
                                        w = wva if c < 4 else wvb
                                        nc.tensor.matmul(
                                            ps[:],
                                            lhsT=xts8[:, 2*c:2*c+2,
                                                      ts(s - 4, P)],
                                            rhs=w[:, (2*c) % 8:(2*c) % 8 + 2,
                                                  :],
                                            start=(c == 0), stop=(c == 7),
                                            perf_mode=DR,
                                        )
                                    sv = stv_pool.tile([P, 512], F16,
                                                       tag="sv")
                                    nc.scalar.activation(sv[:], ps[:], Copy,
                                                         scale=1.0 / W8_SCALE)
                                nc.gpsimd.dma_start(
                                    vv_all[s // 2, s % 2, :, ts(n, 512)],
                                    sv[:])
                        nc.gpsimd.collective_compute(
                            "AllGather", mybir.AluOpType.bypass,
                            replica_groups=PAIRS,
                            ins=[vv_all.opt()], outs=[vgs_all.opt()],
                        )

                        # --- Q (slot0 fp16 -> qt16; slot1 fp8 DR -> qt8)
                        for m in range(DC):
                            if m + 7 < DC:
                                load_wq(m + 7)
                            wt = wq16_tiles.pop(m)
                            ps = ps_all.tile([P, 512], F32, tag="ps")
                            for dc in range(DC):
                                nc.tensor.matmul(
                                    ps[:], lhsT=wt[:, dc, :],
                                    rhs=xts16[:, dc, :],
                                    start=(dc == 0), stop=(dc == DC - 1),
                                )
                            nc.vector.tensor_copy(qt16[:, m, :], ps[:])

                            wt8 = wq8_tiles.pop(m)
                            ps = ps_all.tile([P, 512], F32, tag="ps")
                            for c in range(8):
                                nc.tensor.matmul(
                                    ps[:], lhsT=wt8[:, 2*c:2*c+2, :],
                                    rhs=xts8[:, 2*c:2*c+2, :],
                                    start=(c == 0), stop=(c == 7),
                                    perf_mode=DR,
                                )
                            nc.scalar.activation(qt8[:, m, :], ps[:], Copy,
                                                 scale=QK_SCALE / W8_SCALE)

                # ---------------- phase 2: attention ----------------
                with (
                    tc.tile_pool(name="pt", bufs=1) as pt_pool,
                    tc.tile_pool(name="mk", bufs=1) as mk_pool,
                    tc.tile_pool(name="vb", bufs=2) as vb_pool,
                    tc.tile_pool(name="ktg16", bufs=2) as ktg16_pool,
                    tc.tile_pool(name="ktg8", bufs=3) as ktg8_pool,
                    tc.tile_pool(name="one", bufs=1) as one_pool,
                    tc.tile_pool(name="sc", bufs=4) as sc_pool,
                    tc.tile_pool(name="ob", bufs=3) as ob_pool,
                ):
                    # gathered K tiles: ktg8 on gpsimd (scores s1
                    # runs first), ktg16 split sync/gpsimd
                    ktg8 = {}
                    for r, h in ((0, 0), (1, 0), (0, 1), (1, 1)):
                        kt = ktg8_pool.tile([P, DC, 512], E4, tag="ktg8",
                                            name=f"ktg8_{r}{h}")
                        nc.gpsimd.dma_start(kt[:], kg8[r, h])
                        ktg8[(r, h)] = kt
                    ktg16 = []
                    for r in range(2):
                        kt = ktg16_pool.tile([P, DC, 512], F16, tag="ktg",
                                             name=f"ktg16_{r}")
                        eng = nc.sync if r == 0 else nc.gpsimd
                        eng.dma_start(kt[:], kg16[r])
                        ktg16.append(kt)
                    mk = mk_pool.tile([P, 24, 512], F16, tag="mk")
                    nc.sync.dma_start(mk[:], mk_d.ap())
                    ones = one_pool.tile([P, 1], F16, tag="ones")
                    nc.vector.memset(ones[:], 1.0)
                    # pt: slot0 j -> key pos SLOT0_POS[j]; slot1 kb -> 8+kb
                    pt = pt_pool.tile([P, 24, 512], F16, tag="pt")

                    # --- slot-1 scores (fp8 DoubleRow): 16 key positions,
                    # lo tiles (kg8[.,0]) first, then hi tiles
                    for pos in ([0, 1, 2, 3, 8, 9, 10, 11]
                                + [4, 5, 6, 7, 12, 13, 14, 15]):
                        o = pos % 8
                        kt_t = ktg8[(pos // 8, 0 if o < 4 else 1)]
                        jb = o % 4
                        ps = ps_all.tile([P, 512], F32, tag="ps")
                        for c in range(8):
                            nc.tensor.matmul(
                                ps[:],
                                lhsT=kt_t[:, 2*c:2*c+2, ds(jb * P, P)],
                                rhs=qt8[:, 2*c:2*c+2, :],
                                start=(c == 0), stop=(c == 7),
                                perf_mode=DR,
                            )
                        nc.scalar.activation(pt[:, 8 + pos, :], ps[:], Exp,
                                             scale=INV_SQRT_D / (QK_SCALE ** 2))

                    # --- slot-0 scores (fp16): 8 key positions
                    for pos in SLOT0_POS:
                        kt_t = ktg16[pos // 8]
                        jb = pos % 4
                        ps = ps_all.tile([P, 512], F32, tag="ps")
                        for dc in range(DC):
                            nc.tensor.matmul(
                                ps[:],
                                lhsT=kt_t[:, dc, ds(jb * P, P)],
                                rhs=qt16[:, dc, :],
                                start=(dc == 0), stop=(dc == DC - 1),
                            )
                        nc.scalar.activation(pt[:, S0IDX[pos], :], ps[:],
                                             Exp, scale=INV_SQRT_D)

                    vbig = []
                    for half in range(2):
                        vb = vb_pool.tile([P, 8, D], F16, tag="vb",
                                          name=f"vb{half}")
                        for j in range(8):
                            nc.gpsimd.dma_start(
                                vb[:, j, :], vgs_all[half, j // 2, j % 2])
                        vbig.append(vb)

                    def vt_ap(pos):
                        return vbig[pos // 8][:, pos % 8, :]

                    for j in range(24):
                        nc.vector.tensor_mul(pt[:, j, :], pt[:, j, :],
                                             mk[:, j, :])

                    for slot in range(2):
                        for qs in range(4):
                            # per-qs causal union over the pair
                            if slot == 0:
                                kpos = [0, 1, 2, 3] + list(range(8, 9 + qs))
                                idx = [S0IDX[p] for p in kpos]
                            else:
                                kpos = ([0, 1, 2, 3] + list(range(4, 5 + qs))
                                        + list(range(8, 16)))
                                idx = [8 + p for p in kpos]
                            plt = ps_all.tile([P, 512], F32, tag="ps",
                                              name="pl")
                            pl = plt[:, :1]
                            for i, j in enumerate(idx):
                                nc.tensor.matmul(
                                    pl[:], lhsT=pt[:, j, ts(qs, P)],
                                    rhs=ones[:],
                                    start=(i == 0), stop=(i == len(idx) - 1),
                                )
                            rl = sc_pool.tile([P, 1], F32, tag="rl")
                            nc.vector.reciprocal(rl[:], pl[:])
                            for n in range(4):
                                pav = ps_all.tile([P, 512], F32, tag="ps",
                                                  name="pav")
                                for i, j in enumerate(idx):
                                    nc.tensor.matmul(
                                        pav[:], lhsT=pt[:, j, ts(qs, P)],
                                        rhs=vt_ap(kpos[i])[:, ts(n, 512)],
                                        start=(i == 0),
                                        stop=(i == len(idx) - 1),
                                    )
                                ob = ob_pool.tile([P, 512], F32, tag="ob")
                                if n % 2 == 0:
                                    nc.vector.tensor_scalar_mul(ob[:],
                                                                pav[:],
                                                                rl[:])
                                else:
                                    nc.scalar.activation(ob[:], pav[:],
                                                         Copy, scale=rl[:])
                                eng = nc.sync if n % 2 == 0 else nc.scalar
                                eng.dma_start(
                                    out_d.ap()[ds(slot * 512 + qs * P, P),
                                               ts(n, 512)],
                                    ob[:],
                                )

    nc.compile()
    _CACHED_NC = nc
    return nc


def _host_prep(x, Wq, Wk, Wv):
    """Build per-core input maps (host-side layout prep)."""
    E4np = ml_dtypes.float8_e4m3

    def wqk_prep(W, dt, scale=1.0):
        return np.ascontiguousarray(
            (W * scale).reshape(DC, P, DC, P).transpose(2, 1, 0, 3)).astype(dt)

    def wv_prep(W, dt, scale=1.0):
        return np.ascontiguousarray(
            (W * scale).reshape(2, 8, P, 4, 512).transpose(3, 0, 2, 1, 4)
        ).astype(dt)

    wq16_h = wqk_prep(Wq, np.float16)
    wk16_h = wqk_prep(Wk, np.float16)
    wq8_h = wqk_prep(Wq, E4np, W8_SCALE)
    wk8_h = wqk_prep(Wk, E4np, W8_SCALE)
    wv16_h = wv_prep(Wv, np.float16)
    wv8_h = wv_prep(Wv, E4np, W8_SCALE)

    k_in_block = np.arange(P, dtype=np.int64)[:, None]           # [P, 1]
    q_in_chunk = np.arange(512, dtype=np.int64)[None, :]         # [1, 512]

    def build_masks(h):
        c_lo, c_hi = h, 3 - h
        masks = np.zeros((P, 24, 512), dtype=np.float16)
        for j, pos in enumerate(SLOT0_POS):
            tkb = POS2TRUE[pos // 4] * 4 + pos % 4
            masks[:, j, :] = (tkb * P + k_in_block) <= (c_lo * 512 + q_in_chunk)
        for pos in range(SB):
            tkb = POS2TRUE[pos // 4] * 4 + pos % 4
            masks[:, 8 + pos, :] = (tkb * P + k_in_block) <= (c_hi * 512 + q_in_chunk)
        return masks

    mask_h = [build_masks(0), build_masks(1)]

    def xt_prep(xt_slice, dt):
        # [D, 512] -> [P, DC, 512] (partition-major, one DMA per core)
        return np.ascontiguousarray(
            xt_slice.reshape(DC, P, 512).transpose(1, 0, 2)).astype(dt)

    in_maps = []
    for core in range(NCORES):
        b, h = divmod(core, 2)
        c_lo, c_hi = h, 3 - h
        xt = x[b].T                                               # [D, S] view
        in_maps.append({
            "xt16": xt_prep(xt[:, c_lo * 512:(c_lo + 1) * 512], np.float16),
            "xt8": xt_prep(xt[:, c_hi * 512:(c_hi + 1) * 512], E4np),
            "wq16": wq16_h, "wk16": wk16_h, "wv16": wv16_h,
            "wq8": wq8_h, "wk8": wk8_h, "wv8": wv8_h,
            "masks": mask_h[h],
        })
    return in_maps


def run(x, Wq, Wk, Wv, trace=False):
    x = np.asarray(x, dtype=np.float32)
    Wq = np.asarray(Wq, dtype=np.float32)
    Wk = np.asarray(Wk, dtype=np.float32)
    Wv = np.asarray(Wv, dtype=np.float32)
    nc = build_nc()
    in_maps = _host_prep(x, Wq, Wk, Wv)
    res = run_bass_kernel_spmd(nc, in_maps, core_ids=list(range(NCORES)),
                               trace=trace)
    out = np.empty((B, S, D), dtype=np.float32)
    for core in range(NCORES):
        b, h = divmod(core, 2)
        c_lo, c_hi = h, 3 - h
        o = res.results[core]["out"]
        out[b, c_lo * 512:(c_lo + 1) * 512] = o[:512]
        out[b, c_hi * 512:(c_hi + 1) * 512] = o[512:]
    return out, res


def kernel(x, Wq, Wk, Wv):
    out, _ = run(x, Wq, Wk, Wv)
    return out


if __name__ == "__main__":
    build_nc()
    print("build + compile OK")
